# revision 57
# baseline (speedup 1.0000x reference)
"""ContextualLoss forward on 8 Trainium2 NeuronCores (v3).

Math (reference):
    mu[m]   = mean_c Y[c, m]                      (PONO over channels of Y)
    Xc = X - mu ; Yc = Y - mu                     (both centered by Y's mean)
    cos[i,j] = <Xc_i, Yc_j> / (|Xc_i| |Yc_j|)
    d = 1 - cos ; dn = d / (min_j d + 1e-3) ; w = exp((1 - dn)/0.1)
    A = w / sum_j w ; CX_b = mean_i max_j A ; loss = mean_b -log CX_b

Device-side structure (per core: one sample b, one 2048-row half):
  * Inputs are converted to fp16 on the host: halves DMA traffic and fp16
    matmuls run at full PE rate.
  * Y is centered and column-scaled in place: Yn = (Y - mu) / |Yc|, so the
    big matmul produces G[i,j] = cos[i,j] * |Xc_i| directly in PSUM.
    (X never needs centering for the matmul: Yn has zero channel-sum.)
  * mu and |Yc|^2 are broadcast to [128, M] via "fat ones" matmuls
    (lhsT = 1/256 resp. 1 in every entry), skipping row-copy round trips.
  * |Xc_i|^2 = QX_i - sy_i*SX_i/128 + sy_i^2/256 from per-row raw stats
    (tile-major [128,16] via tiny N=1 matmuls), so X is never modified.
  * The fused DVE drain tensor_scalar(scalar1=1/|Xc_i| per partition,
    op0=mult, op1=max, accum_out) moves each PSUM half-tile to fp16 SBUF as
    d = cos AND computes the running row max in the same single scan; the h1
    pass seeds the max with h0's result via the scalar2 initializer. This
    scan (1 elem/cycle/lane; DVE is the only engine that can reduce PSUM) is
    the kernel's bottleneck: ~72us of the ~110us total.
  * ScalarE does only Exp over the drained d tiles (scale=10r, bias=10-10.01r
    fold the softmin exponent; accum_out gives sum_j w). PE p-state warmup
    matmuls and an early dummy rsqrt keep table loads off the critical path.
  * max_j A = exp(0.01/(dmin+1e-3)) / sum_j w analytically (no second pass).

Engine busy per core: DVE ~85us (drain+max scans, y centering/scaling),
Act ~80us (exp stream + rsqrts), PE ~64us (matmuls), DMA ~9us (fp16 in).

Sharding: core c -> sample b = c//2, row-half h = c%2 (2048 rows each).
Each core's Y is column-permuted host-side to [own-half | other-half] so the
identical SPMD program can read the X-half's stats from columns [0, 2048).
Row reductions are permutation-invariant, so the permutation is harmless.
"""

import sys

sys.path.insert(0, "/opt/trn_rl_repo")

import numpy as np

import concourse.bass as bass
import concourse.tile as tile
from concourse import bacc
from concourse import mybir
from concourse.bass_utils import run_bass_kernel_spmd

B = 4
C = 256
M = 4096  # 64*64 spatial positions
HALF = M // 2  # rows per core
NT = HALF // 128  # 16 i-tiles per core
N_CORES = 8
Q = 1024  # preprocessing quarter width
HW = 2048  # main-loop psum half width

F32 = mybir.dt.float32
F16 = mybir.dt.float16
AF = mybir.ActivationFunctionType
ALU = mybir.AluOpType


def build_nc() -> bass.Bass:
    nc = bacc.Bacc()

    x_d = nc.declare_dram_parameter("x", [C, HALF], F16, isOutput=False)
    y_d = nc.declare_dram_parameter("y", [C, M], F16, isOutput=False)
    v_d = nc.declare_dram_parameter("v", [128, NT], F32, isOutput=True)

    x_v = x_d.rearrange("(k p) m -> p k m", p=128)
    y_v = y_d.rearrange("(k p) m -> p k m", p=128)

    with tile.TileContext(nc) as tc:
        with (
            tc.tile_pool(name="io", bufs=1) as io,
            tc.tile_pool(name="consts", bufs=1) as consts,
            tc.tile_pool(name="stats", bufs=1) as stats,
            tc.tile_pool(name="dpool", bufs=8) as dpool,
            tc.tile_pool(name="wpool", bufs=1) as wpool,
        ):
            y_sb = io.tile([128, 2, M], F16)
            x_sb = io.tile([128, 2, HALF], F16)

            ones512 = consts.tile([128, 512], F16)
            nc.vector.memset(ones512, 1.0)
            ones_mat = consts.tile([128, 128], F16)
            nc.vector.memset(ones_mat, 1.0)
            inv256_mat = consts.tile([128, 128], F16)
            nc.vector.memset(inv256_mat, 1.0 / 256.0)
            ones_col = consts.tile([128, 1], F16)
            nc.vector.memset(ones_col, 1.0)

            sy16 = stats.tile([128, NT], F32)
            sx16 = stats.tile([128, NT], F32)
            qx16 = stats.tile([128, NT], F32)
            nx2 = stats.tile([128, NT], F32)
            inv_nx = stats.tile([128, NT], F32)
            cmaxA = stats.tile([128, NT], F32)
            cmaxB = stats.tile([128, NT], F32)
            u16 = stats.tile([128, NT], F32)
            r16 = stats.tile([128, NT], F32)
            scale16 = stats.tile([128, NT], F32)
            bias16 = stats.tile([128, NT], F32)
            sumw16 = stats.tile([128, NT], F32)
            maxw16 = stats.tile([128, NT], F32)
            rs16 = stats.tile([128, NT], F32)
            v16 = stats.tile([128, NT], F32)

            # One shared PSUM pool: the same [128, 2048] ring serves the
            # preprocessing broadcasts AND the main G half-tiles, so early
            # tiles' matmuls start while half B is still being prepared.
            with (
                tc.tile_pool(name="psA", bufs=2, space="PSUM") as psA,
                tc.tile_pool(name="presb", bufs=1) as presb,
                tc.tile_pool(name="sqpool", bufs=2) as sqpool,
            ):
                mu_b = presb.tile([128, M], F16)
                inv_ny = presb.tile([128, M], F16)
                sqx = presb.tile([128, 2, HALF], F16)
                w_sb = wpool.tile([128, M], F16)
                dummy1 = presb.tile([128, 1], F32)
                nc.vector.memset(dummy1, 1.0)
                dummy2 = presb.tile([128, 1], F32)
                # Preload the rsqrt activation table while DMA runs, so the
                # load is off the preprocessing critical path.
                nc.scalar.activation(dummy2[:, :], dummy1[:, :],
                                     AF.Abs_reciprocal_sqrt)


                def quarter_mm(ps, q, lhsT, src, h):
                    # ps[:, q-slice][p, j] = sum_c lhsT[c, p]*src[c, j] (K=256)
                    for k in range(2):
                        for j in range(2):
                            lo = q * Q + j * 512
                            nc.tensor.matmul(
                                ps[:, lo : lo + 512],
                                lhsT=lhsT[:, :],
                                rhs=src[:, k, h * HW + lo : h * HW + lo + 512],
                                start=(k == 0),
                                stop=(k == 1),
                            )

                def stat16(dst, src_tile, ps, off):
                    # dst[p, t] = sum_c src[c, t*128+p] via N=1 matmuls; the
                    # three row-stat groups share one PSUM ring slot (disjoint
                    # column ranges) so they cost one rotation, not three
                    for t in range(NT):
                        for k in range(2):
                            nc.tensor.matmul(
                                ps[:, off + t : off + t + 1],
                                lhsT=src_tile[:, k, t * 128 : (t + 1) * 128],
                                rhs=ones_col[:, :],
                                start=(k == 0),
                                stop=(k == 1),
                            )
                    nc.vector.tensor_copy(dst[:, :], ps[:, off : off + NT])

                def cp_q(ps, h, q):
                    lo = h * HW + q * Q
                    nc.scalar.copy(mu_b[:, lo : lo + Q], ps[:, q * Q : (q + 1) * Q])

                def sub_q(h, q):
                    lo = h * HW + q * Q
                    for k in range(2):
                        nc.vector.tensor_sub(
                            y_sb[:, k, lo : lo + Q],
                            y_sb[:, k, lo : lo + Q],
                            mu_b[:, lo : lo + Q],
                        )

                def sq_q(sq, h, q):
                    lo = h * HW + q * Q
                    nc.scalar.activation(
                        sq[:, :, q * Q : (q + 1) * Q],
                        y_sb[:, :, lo : lo + Q],
                        AF.Square,
                    )

                def rsq_q(ps, h, q):
                    lo = h * HW + q * Q
                    nc.scalar.activation(
                        inv_ny[:, lo : lo + Q], ps[:, q * Q : (q + 1) * Q],
                        AF.Abs_reciprocal_sqrt,
                    )

                def mul_q(h, q):
                    lo = h * HW + q * Q
                    for k in range(2):
                        nc.vector.tensor_mul(
                            y_sb[:, k, lo : lo + Q],
                            y_sb[:, k, lo : lo + Q],
                            inv_ny[:, lo : lo + Q],
                        )

                def g_half(t, h, d_t):
                    ps = psA.tile([128, HW], F32, tag="g")
                    for k in range(2):
                        for j in range(4):
                            nc.tensor.matmul(
                                ps[:, j * 512 : (j + 1) * 512],
                                lhsT=x_sb[:, k, t * 128 : (t + 1) * 128],
                                rhs=y_sb[:, k, h * HW + j * 512 : h * HW + (j + 1) * 512],
                                start=(k == 0),
                                stop=(k == 1),
                            )
                    # Fused drain: d = G * (1/|Xc_i|) = cos (fp16), and the
                    # accumulator computes the row max; the h1 pass seeds the
                    # running max with h0's result via the scalar2 initializer.
                    if h == 0:
                        nc.vector.tensor_scalar(
                            out=d_t[:, 0:HW],
                            in0=ps[:, :],
                            scalar1=inv_nx[:, t : t + 1],
                            scalar2=None,
                            op0=ALU.mult,
                            op1=ALU.max,
                            accum_out=cmaxA[:, t : t + 1],
                        )
                    else:
                        nc.vector.tensor_scalar(
                            out=d_t[:, HW:M],
                            in0=ps[:, :],
                            scalar1=inv_nx[:, t : t + 1],
                            scalar2=cmaxA[:, t : t + 1],
                            op0=ALU.mult,
                            op1=ALU.max,
                            accum_out=cmaxB[:, t : t + 1],
                        )

                def stats_a(t):
                    # emitted right after drain(t, h1): u = 1.001 - cosmax
                    sl = slice(t, t + 1)
                    nc.gpsimd.tensor_scalar(
                        out=u16[:, sl], in0=cmaxB[:, sl],
                        scalar1=-1.0, scalar2=1.001, op0=ALU.mult, op1=ALU.add,
                    )

                def stats_b(t):
                    # r = 1/u via Pool's normalize_recip (in=1.0, denom=u), so
                    # the whole stats chain stays off the DVE drain stream
                    sl = slice(t, t + 1)
                    nc.gpsimd.normalize_recip(
                        out_ap=r16[:, sl], in_ap=dummy1[:, :], denom_ap=u16[:, sl]
                    )
                    nc.gpsimd.tensor_scalar_mul(scale16[:, sl], r16[:, sl], 10.0)
                    nc.gpsimd.tensor_scalar(
                        out=bias16[:, sl], in0=r16[:, sl],
                        scalar1=-10.0, scalar2=10.0, op0=ALU.mult, op1=ALU.add,
                    )

                def exp_tile(t, d_t):
                    nc.scalar.activation(
                        out=w_sb[:, :],
                        in_=d_t[:, :],
                        func=AF.Exp,
                        bias=bias16[:, t : t + 1],
                        scale=scale16[:, t : t + 1],
                        accum_out=sumw16[:, t : t + 1],
                    )

                # Warm up the PE p-state with dummy matmuls during DMA wait.
                wu_ps = psA.tile([128, HW], F32, tag="g")
                for i in range(4):
                    nc.tensor.matmul(
                        wu_ps[:, 0:512],
                        lhsT=ones_mat[:, :],
                        rhs=ones512[:, :],
                        start=True,
                        stop=True,
                    )

                # DMA: y half A quarters, x, y half B quarters
                nc.sync.dma_start(out=y_sb[:, :, 0:Q], in_=y_v[:, :, 0:Q])
                nc.sync.dma_start(out=y_sb[:, :, Q:HW], in_=y_v[:, :, Q:HW])
                nc.sync.dma_start(out=x_sb[:, :, :], in_=x_v[:, :, :])
                nc.sync.dma_start(out=y_sb[:, :, HW : HW + Q], in_=y_v[:, :, HW : HW + Q])
                nc.sync.dma_start(out=y_sb[:, :, HW + Q : M], in_=y_v[:, :, HW + Q : M])

                d_tiles = {}

                def new_d(t):
                    d_tiles[t] = dpool.tile([128, M], F16, tag="d", name=f"d{t}")
                    return d_tiles[t]

                # --- preprocessing; emission order = engine-queue order,
                # chosen so no queue head-of-line blocks for long
                mu0_ps = psA.tile([128, HW], F32, tag="g", name="mu0")
                quarter_mm(mu0_ps, 0, inv256_mat, y_sb, 0)
                cp_q(mu0_ps, 0, 0)
                quarter_mm(mu0_ps, 1, inv256_mat, y_sb, 0)
                cp_q(mu0_ps, 0, 1)
                st_ps = psA.tile([128, HW], F32, tag="g", name="st")
                stat16(sy16, y_sb, st_ps, 0)  # reads only columns [0, 2048)
                sub_q(0, 0)
                sq0 = sqpool.tile([128, 2, HW], F16, tag="sq", name="sq0")
                sq_q(sq0, 0, 0)
                sub_q(0, 1)
                sq_q(sq0, 0, 1)
                stat16(sx16, x_sb, st_ps, 16)
                nc.vector.tensor_mul(sqx[:, :, :], x_sb[:, :, :], x_sb[:, :, :])
                mu1_ps = psA.tile([128, HW], F32, tag="g", name="mu1")
                quarter_mm(mu1_ps, 0, inv256_mat, y_sb, 1)
                cp_q(mu1_ps, 1, 0)
                quarter_mm(mu1_ps, 1, inv256_mat, y_sb, 1)
                cp_q(mu1_ps, 1, 1)
                qy0_ps = psA.tile([128, HW], F32, tag="g", name="qy0")
                quarter_mm(qy0_ps, 0, ones_mat, sq0, 0)
                rsq_q(qy0_ps, 0, 0)
                quarter_mm(qy0_ps, 1, ones_mat, sq0, 0)
                rsq_q(qy0_ps, 0, 1)
                mul_q(0, 0)
                mul_q(0, 1)
                stat16(qx16, sqx, st_ps, 32)
                # nx2 = qx - sy*sx/128 + sy^2/256  (Pool, feeding inv_nx which
                # the fused drains now need early)
                t1 = stats.tile([128, NT], F32)
                t2 = stats.tile([128, NT], F32)
                nc.gpsimd.tensor_scalar_mul(t1[:, :], sy16[:, :], -1.0 / 128.0)
                nc.gpsimd.tensor_mul(t1[:, :], t1[:, :], sx16[:, :])
                nc.gpsimd.tensor_add(nx2[:, :], qx16[:, :], t1[:, :])
                nc.gpsimd.tensor_scalar_mul(t2[:, :], sy16[:, :], 1.0 / 256.0)
                nc.gpsimd.tensor_mul(t2[:, :], t2[:, :], sy16[:, :])
                nc.gpsimd.tensor_add(nx2[:, :], nx2[:, :], t2[:, :])
                nc.scalar.activation(inv_nx[:, :], nx2[:, :], AF.Abs_reciprocal_sqrt)
                # --- half B chain
                sub_q(1, 0)
                sq1 = sqpool.tile([128, 2, HW], F16, tag="sq", name="sq1")
                sq_q(sq1, 1, 0)
                sub_q(1, 1)
                sq_q(sq1, 1, 1)
                qy1_ps = psA.tile([128, HW], F32, tag="g", name="qy1")
                quarter_mm(qy1_ps, 0, ones_mat, sq1, 0)
                rsq_q(qy1_ps, 1, 0)
                quarter_mm(qy1_ps, 1, ones_mat, sq1, 0)
                rsq_q(qy1_ps, 1, 1)
                mul_q(1, 0)
                mul_q(1, 1)
                g_half(0, 0, new_d(0))
                g_half(1, 0, new_d(1))
                g_half(2, 0, new_d(2))
                g_half(3, 0, new_d(3))

                # --- steady state: drain h1 of tile t, then lag the recip by
                # one drain (h0 of tile t+4) so DVE never waits on Pool
                done_h0 = 4
                for t in range(NT):
                    g_half(t, 1, d_tiles[t])
                    stats_a(t)
                    # skip the h0-ahead on the first two iterations so h0
                    # work lasts until t=13: the tail then alternates h0/h1
                    # drains and Act's exp stream packs instead of backlogging
                    tn = done_h0
                    emit_h0 = tn < NT and t >= 2
                    if emit_h0 and (t >= 10 or t <= 1):
                        stats_b(t)
                        g_half(tn, 0, new_d(tn))
                        done_h0 += 1
                    elif emit_h0:
                        g_half(tn, 0, new_d(tn))
                        done_h0 += 1
                        stats_b(t)
                    else:
                        stats_b(t)
                    if t == NT - 1:
                        # all tail work that only needs tiles 0..14 runs here,
                        # hidden behind the last exps instead of trailing them
                        nc.scalar.activation(
                            maxw16[:, :], r16[:, :], AF.Exp, scale=0.01
                        )
                        nc.vector.reciprocal(
                            rs16[:, 0 : NT - 1], sumw16[:, 0 : NT - 1]
                        )
                        nc.gpsimd.tensor_mul(
                            v16[:, 0 : NT - 1], maxw16[:, 0 : NT - 1],
                            rs16[:, 0 : NT - 1],
                        )
                        nc.sync.dma_start(
                            out=v_d[:, 0 : NT - 1], in_=v16[:, 0 : NT - 1]
                        )
                    exp_tile(t, d_tiles.pop(t))

                # ---- epilogue: v = exp(0.01*r) / sumw ------------------
                nc.vector.reciprocal(rs16[:, NT - 1 : NT], sumw16[:, NT - 1 : NT])
                nc.gpsimd.tensor_mul(
                    v16[:, NT - 1 : NT], maxw16[:, NT - 1 : NT],
                    rs16[:, NT - 1 : NT],
                )
                nc.sync.dma_start(
                    out=v_d[:, NT - 1 : NT], in_=v16[:, NT - 1 : NT]
                )

    nc.compile()
    return nc


_NC = None


def _get_nc():
    global _NC
    if _NC is None:
        _NC = build_nc()
    return _NC


def make_in_maps(X, Y):
    """Per-core fp16 inputs. Y columns permuted to [own-half | other-half]."""
    in_maps = []
    for c in range(N_CORES):
        b, h = c // 2, c % 2
        xs = np.ascontiguousarray(X[b][:, h * HALF : (h + 1) * HALF]).astype(
            np.float16
        )
        ys = np.ascontiguousarray(
            np.concatenate(
                [
                    Y[b][:, h * HALF : (h + 1) * HALF],
                    Y[b][:, (1 - h) * HALF : (2 - h) * HALF],
                ],
                axis=1,
            )
        ).astype(np.float16)
        in_maps.append({"x": xs, "y": ys})
    return in_maps


def finish_host(results):
    """results: list of 8 per-core dicts with 'v' [128, NT]."""
    cx = np.zeros(B, dtype=np.float64)
    for c in range(N_CORES):
        cx[c // 2] += results[c]["v"].astype(np.float64).sum()
    cx /= M
    return np.float32(np.mean(-np.log(cx)))


def run(X_features, Y_features, trace=False, tmpdir=None):
    X = np.asarray(X_features, dtype=np.float32).reshape(B, C, M)
    Y = np.asarray(Y_features, dtype=np.float32).reshape(B, C, M)
    nc = _get_nc()
    res = run_bass_kernel_spmd(
        nc, make_in_maps(X, Y), list(range(N_CORES)), trace=trace, tmpdir=tmpdir
    )
    return finish_host(res.results), res


def kernel(X_features, Y_features):
    loss, _ = run(X_features, Y_features)
    return loss


# revision 58
# speedup vs baseline: 1.0031x; 1.0031x over previous
"""ContextualLoss forward on 8 Trainium2 NeuronCores (v3).

Math (reference):
    mu[m]   = mean_c Y[c, m]                      (PONO over channels of Y)
    Xc = X - mu ; Yc = Y - mu                     (both centered by Y's mean)
    cos[i,j] = <Xc_i, Yc_j> / (|Xc_i| |Yc_j|)
    d = 1 - cos ; dn = d / (min_j d + 1e-3) ; w = exp((1 - dn)/0.1)
    A = w / sum_j w ; CX_b = mean_i max_j A ; loss = mean_b -log CX_b

Device-side structure (per core: one sample b, one 2048-row half):
  * Inputs are converted to fp16 on the host: halves DMA traffic and fp16
    matmuls run at full PE rate.
  * Y is centered and column-scaled in place: Yn = (Y - mu) / |Yc|, so the
    big matmul produces G[i,j] = cos[i,j] * |Xc_i| directly in PSUM.
    (X never needs centering for the matmul: Yn has zero channel-sum.)
  * mu and |Yc|^2 are broadcast to [128, M] via "fat ones" matmuls
    (lhsT = 1/256 resp. 1 in every entry), skipping row-copy round trips.
  * |Xc_i|^2 = QX_i - sy_i*SX_i/128 + sy_i^2/256 from per-row raw stats
    (tile-major [128,16] via tiny N=1 matmuls), so X is never modified.
  * The fused DVE drain tensor_scalar(scalar1=1/|Xc_i| per partition,
    op0=mult, op1=max, accum_out) moves each PSUM half-tile to fp16 SBUF as
    d = cos AND computes the running row max in the same single scan; the h1
    pass seeds the max with h0's result via the scalar2 initializer. This
    scan (1 elem/cycle/lane; DVE is the only engine that can reduce PSUM) is
    the kernel's bottleneck: ~72us of the ~110us total.
  * ScalarE does only Exp over the drained d tiles (scale=10r, bias=10-10.01r
    fold the softmin exponent; accum_out gives sum_j w). PE p-state warmup
    matmuls and an early dummy rsqrt keep table loads off the critical path.
  * max_j A = exp(0.01/(dmin+1e-3)) / sum_j w analytically (no second pass).

Engine busy per core: DVE ~85us (drain+max scans, y centering/scaling),
Act ~80us (exp stream + rsqrts), PE ~64us (matmuls), DMA ~9us (fp16 in).

Sharding: core c -> sample b = c//2, row-half h = c%2 (2048 rows each).
Each core's Y is column-permuted host-side to [own-half | other-half] so the
identical SPMD program can read the X-half's stats from columns [0, 2048).
Row reductions are permutation-invariant, so the permutation is harmless.
"""

import sys

sys.path.insert(0, "/opt/trn_rl_repo")

import numpy as np

import concourse.bass as bass
import concourse.tile as tile
from concourse import bacc
from concourse import mybir
from concourse.bass_utils import run_bass_kernel_spmd

B = 4
C = 256
M = 4096  # 64*64 spatial positions
HALF = M // 2  # rows per core
NT = HALF // 128  # 16 i-tiles per core
N_CORES = 8
Q = 1024  # preprocessing quarter width
HW = 2048  # main-loop psum half width

F32 = mybir.dt.float32
F16 = mybir.dt.float16
AF = mybir.ActivationFunctionType
ALU = mybir.AluOpType


def build_nc() -> bass.Bass:
    nc = bacc.Bacc()

    x_d = nc.declare_dram_parameter("x", [C, HALF], F16, isOutput=False)
    y_d = nc.declare_dram_parameter("y", [C, M], F16, isOutput=False)
    v_d = nc.declare_dram_parameter("v", [128, NT], F32, isOutput=True)

    x_v = x_d.rearrange("(k p) m -> p k m", p=128)
    y_v = y_d.rearrange("(k p) m -> p k m", p=128)

    with tile.TileContext(nc) as tc:
        with (
            tc.tile_pool(name="io", bufs=1) as io,
            tc.tile_pool(name="consts", bufs=1) as consts,
            tc.tile_pool(name="stats", bufs=1) as stats,
            tc.tile_pool(name="dpool", bufs=8) as dpool,
            tc.tile_pool(name="wpool", bufs=1) as wpool,
        ):
            y_sb = io.tile([128, 2, M], F16)
            x_sb = io.tile([128, 2, HALF], F16)

            ones512 = consts.tile([128, 512], F16)
            nc.vector.memset(ones512, 1.0)
            ones_mat = consts.tile([128, 128], F16)
            nc.vector.memset(ones_mat, 1.0)
            inv256_mat = consts.tile([128, 128], F16)
            nc.vector.memset(inv256_mat, 1.0 / 256.0)
            ones_col = consts.tile([128, 1], F16)
            nc.vector.memset(ones_col, 1.0)

            sy16 = stats.tile([128, NT], F32)
            sx16 = stats.tile([128, NT], F32)
            qx16 = stats.tile([128, NT], F32)
            nx2 = stats.tile([128, NT], F32)
            inv_nx = stats.tile([128, NT], F32)
            cmaxA = stats.tile([128, NT], F32)
            cmaxB = stats.tile([128, NT], F32)
            u16 = stats.tile([128, NT], F32)
            r16 = stats.tile([128, NT], F32)
            scale16 = stats.tile([128, NT], F32)
            bias16 = stats.tile([128, NT], F32)
            sumw16 = stats.tile([128, NT], F32)
            maxw16 = stats.tile([128, NT], F32)
            rs16 = stats.tile([128, NT], F32)
            v16 = stats.tile([128, NT], F32)

            # One shared PSUM pool: the same [128, 2048] ring serves the
            # preprocessing broadcasts AND the main G half-tiles, so early
            # tiles' matmuls start while half B is still being prepared.
            with (
                tc.tile_pool(name="psA", bufs=2, space="PSUM") as psA,
                tc.tile_pool(name="presb", bufs=1) as presb,
                tc.tile_pool(name="sqpool", bufs=2) as sqpool,
            ):
                mu_b = presb.tile([128, M], F16)
                inv_ny = presb.tile([128, M], F16)
                sqx = presb.tile([128, 2, HALF], F16)
                w_sb = wpool.tile([128, M], F16)
                dummy1 = presb.tile([128, 1], F32)
                nc.vector.memset(dummy1, 1.0)
                dummy2 = presb.tile([128, 1], F32)
                # Preload the rsqrt activation table while DMA runs, so the
                # load is off the preprocessing critical path.
                nc.scalar.activation(dummy2[:, :], dummy1[:, :],
                                     AF.Abs_reciprocal_sqrt)


                def quarter_mm(ps, q, lhsT, src, h):
                    # ps[:, q-slice][p, j] = sum_c lhsT[c, p]*src[c, j] (K=256)
                    for k in range(2):
                        for j in range(2):
                            lo = q * Q + j * 512
                            nc.tensor.matmul(
                                ps[:, lo : lo + 512],
                                lhsT=lhsT[:, :],
                                rhs=src[:, k, h * HW + lo : h * HW + lo + 512],
                                start=(k == 0),
                                stop=(k == 1),
                            )

                def stat16(dst, src_tile):
                    # dst[p, t] = sum_c src[c, t*128+p] via N=1 matmuls
                    ps = psA.tile([128, HW], F32, tag="g")
                    for t in range(NT):
                        for k in range(2):
                            nc.tensor.matmul(
                                ps[:, t : t + 1],
                                lhsT=src_tile[:, k, t * 128 : (t + 1) * 128],
                                rhs=ones_col[:, :],
                                start=(k == 0),
                                stop=(k == 1),
                            )
                    nc.vector.tensor_copy(dst[:, :], ps[:, 0:NT])

                def cp_q(ps, h, q):
                    lo = h * HW + q * Q
                    nc.scalar.copy(mu_b[:, lo : lo + Q], ps[:, q * Q : (q + 1) * Q])

                def sub_q(h, q):
                    lo = h * HW + q * Q
                    for k in range(2):
                        nc.vector.tensor_sub(
                            y_sb[:, k, lo : lo + Q],
                            y_sb[:, k, lo : lo + Q],
                            mu_b[:, lo : lo + Q],
                        )

                def sq_q(sq, h, q):
                    lo = h * HW + q * Q
                    nc.scalar.activation(
                        sq[:, :, q * Q : (q + 1) * Q],
                        y_sb[:, :, lo : lo + Q],
                        AF.Square,
                    )

                def rsq_q(ps, h, q):
                    lo = h * HW + q * Q
                    nc.scalar.activation(
                        inv_ny[:, lo : lo + Q], ps[:, q * Q : (q + 1) * Q],
                        AF.Abs_reciprocal_sqrt,
                    )

                def mul_q(h, q):
                    lo = h * HW + q * Q
                    for k in range(2):
                        nc.vector.tensor_mul(
                            y_sb[:, k, lo : lo + Q],
                            y_sb[:, k, lo : lo + Q],
                            inv_ny[:, lo : lo + Q],
                        )

                def g_half(t, h, d_t):
                    ps = psA.tile([128, HW], F32, tag="g")
                    for k in range(2):
                        for j in range(4):
                            nc.tensor.matmul(
                                ps[:, j * 512 : (j + 1) * 512],
                                lhsT=x_sb[:, k, t * 128 : (t + 1) * 128],
                                rhs=y_sb[:, k, h * HW + j * 512 : h * HW + (j + 1) * 512],
                                start=(k == 0),
                                stop=(k == 1),
                            )
                    # Fused drain: d = G * (1/|Xc_i|) = cos (fp16), and the
                    # accumulator computes the row max; the h1 pass seeds the
                    # running max with h0's result via the scalar2 initializer.
                    if h == 0:
                        nc.vector.tensor_scalar(
                            out=d_t[:, 0:HW],
                            in0=ps[:, :],
                            scalar1=inv_nx[:, t : t + 1],
                            scalar2=None,
                            op0=ALU.mult,
                            op1=ALU.max,
                            accum_out=cmaxA[:, t : t + 1],
                        )
                    else:
                        nc.vector.tensor_scalar(
                            out=d_t[:, HW:M],
                            in0=ps[:, :],
                            scalar1=inv_nx[:, t : t + 1],
                            scalar2=cmaxA[:, t : t + 1],
                            op0=ALU.mult,
                            op1=ALU.max,
                            accum_out=cmaxB[:, t : t + 1],
                        )

                def stats_a(t):
                    # emitted right after drain(t, h1): u = 1.001 - cosmax
                    sl = slice(t, t + 1)
                    nc.gpsimd.tensor_scalar(
                        out=u16[:, sl], in0=cmaxB[:, sl],
                        scalar1=-1.0, scalar2=1.001, op0=ALU.mult, op1=ALU.add,
                    )

                def stats_b(t):
                    # r = 1/u via Pool's normalize_recip (in=1.0, denom=u), so
                    # the whole stats chain stays off the DVE drain stream
                    sl = slice(t, t + 1)
                    nc.gpsimd.normalize_recip(
                        out_ap=r16[:, sl], in_ap=dummy1[:, :], denom_ap=u16[:, sl]
                    )
                    nc.gpsimd.tensor_scalar_mul(scale16[:, sl], r16[:, sl], 10.0)
                    nc.gpsimd.tensor_scalar(
                        out=bias16[:, sl], in0=r16[:, sl],
                        scalar1=-10.0, scalar2=10.0, op0=ALU.mult, op1=ALU.add,
                    )

                def exp_tile(t, d_t):
                    nc.scalar.activation(
                        out=w_sb[:, :],
                        in_=d_t[:, :],
                        func=AF.Exp,
                        bias=bias16[:, t : t + 1],
                        scale=scale16[:, t : t + 1],
                        accum_out=sumw16[:, t : t + 1],
                    )

                # Warm up the PE p-state with dummy matmuls during DMA wait.
                wu_ps = psA.tile([128, HW], F32, tag="g")
                for i in range(4):
                    nc.tensor.matmul(
                        wu_ps[:, 0:512],
                        lhsT=ones_mat[:, :],
                        rhs=ones512[:, :],
                        start=True,
                        stop=True,
                    )

                # DMA: y half A quarters, x, y half B quarters
                nc.sync.dma_start(out=y_sb[:, :, 0:Q], in_=y_v[:, :, 0:Q])
                nc.sync.dma_start(out=y_sb[:, :, Q:HW], in_=y_v[:, :, Q:HW])
                nc.sync.dma_start(out=x_sb[:, :, :], in_=x_v[:, :, :])
                nc.sync.dma_start(out=y_sb[:, :, HW : HW + Q], in_=y_v[:, :, HW : HW + Q])
                nc.sync.dma_start(out=y_sb[:, :, HW + Q : M], in_=y_v[:, :, HW + Q : M])

                d_tiles = {}

                def new_d(t):
                    d_tiles[t] = dpool.tile([128, M], F16, tag="d", name=f"d{t}")
                    return d_tiles[t]

                # --- preprocessing; emission order = engine-queue order,
                # chosen so no queue head-of-line blocks for long
                mu0_ps = psA.tile([128, HW], F32, tag="g", name="mu0")
                quarter_mm(mu0_ps, 0, inv256_mat, y_sb, 0)
                cp_q(mu0_ps, 0, 0)
                quarter_mm(mu0_ps, 1, inv256_mat, y_sb, 0)
                cp_q(mu0_ps, 0, 1)
                stat16(sy16, y_sb)  # t-loop reads only columns [0, 2048)
                sub_q(0, 0)
                sq0 = sqpool.tile([128, 2, HW], F16, tag="sq", name="sq0")
                sq_q(sq0, 0, 0)
                sub_q(0, 1)
                sq_q(sq0, 0, 1)
                stat16(sx16, x_sb)
                nc.vector.tensor_mul(sqx[:, :, :], x_sb[:, :, :], x_sb[:, :, :])
                mu1_ps = psA.tile([128, HW], F32, tag="g", name="mu1")
                quarter_mm(mu1_ps, 0, inv256_mat, y_sb, 1)
                cp_q(mu1_ps, 1, 0)
                quarter_mm(mu1_ps, 1, inv256_mat, y_sb, 1)
                cp_q(mu1_ps, 1, 1)
                qy0_ps = psA.tile([128, HW], F32, tag="g", name="qy0")
                quarter_mm(qy0_ps, 0, ones_mat, sq0, 0)
                rsq_q(qy0_ps, 0, 0)
                quarter_mm(qy0_ps, 1, ones_mat, sq0, 0)
                rsq_q(qy0_ps, 0, 1)
                mul_q(0, 0)
                mul_q(0, 1)
                stat16(qx16, sqx)
                # nx2 = qx - sy*sx/128 + sy^2/256  (Pool, feeding inv_nx which
                # the fused drains now need early)
                t1 = stats.tile([128, NT], F32)
                t2 = stats.tile([128, NT], F32)
                nc.gpsimd.tensor_scalar_mul(t1[:, :], sy16[:, :], -1.0 / 128.0)
                nc.gpsimd.tensor_mul(t1[:, :], t1[:, :], sx16[:, :])
                nc.gpsimd.tensor_add(nx2[:, :], qx16[:, :], t1[:, :])
                nc.gpsimd.tensor_scalar_mul(t2[:, :], sy16[:, :], 1.0 / 256.0)
                nc.gpsimd.tensor_mul(t2[:, :], t2[:, :], sy16[:, :])
                nc.gpsimd.tensor_add(nx2[:, :], nx2[:, :], t2[:, :])
                nc.scalar.activation(inv_nx[:, :], nx2[:, :], AF.Abs_reciprocal_sqrt)
                # --- half B chain
                sub_q(1, 0)
                sq1 = sqpool.tile([128, 2, HW], F16, tag="sq", name="sq1")
                sq_q(sq1, 1, 0)
                sub_q(1, 1)
                sq_q(sq1, 1, 1)
                qy1_ps = psA.tile([128, HW], F32, tag="g", name="qy1")
                quarter_mm(qy1_ps, 0, ones_mat, sq1, 0)
                rsq_q(qy1_ps, 1, 0)
                quarter_mm(qy1_ps, 1, ones_mat, sq1, 0)
                rsq_q(qy1_ps, 1, 1)
                mul_q(1, 0)
                mul_q(1, 1)
                g_half(0, 0, new_d(0))
                g_half(1, 0, new_d(1))
                g_half(2, 0, new_d(2))
                g_half(3, 0, new_d(3))

                # --- steady state: drain h1 of tile t, then lag the recip by
                # one drain (h0 of tile t+4) so DVE never waits on Pool
                done_h0 = 4
                for t in range(NT):
                    g_half(t, 1, d_tiles[t])
                    stats_a(t)
                    # skip the h0-ahead on the first two iterations so h0
                    # work lasts until t=13: the tail then alternates h0/h1
                    # drains and Act's exp stream packs instead of backlogging
                    tn = done_h0
                    emit_h0 = tn < NT and t >= 2
                    if emit_h0 and (t >= 10 or t <= 1):
                        stats_b(t)
                        g_half(tn, 0, new_d(tn))
                        done_h0 += 1
                    elif emit_h0:
                        g_half(tn, 0, new_d(tn))
                        done_h0 += 1
                        stats_b(t)
                    else:
                        stats_b(t)
                    if t == NT - 1:
                        # all tail work that only needs tiles 0..14 runs here,
                        # hidden behind the last exps instead of trailing them
                        nc.scalar.activation(
                            maxw16[:, :], r16[:, :], AF.Exp, scale=0.01
                        )
                        nc.vector.reciprocal(
                            rs16[:, 0 : NT - 1], sumw16[:, 0 : NT - 1]
                        )
                        nc.gpsimd.tensor_mul(
                            v16[:, 0 : NT - 1], maxw16[:, 0 : NT - 1],
                            rs16[:, 0 : NT - 1],
                        )
                        nc.sync.dma_start(
                            out=v_d[:, 0 : NT - 1], in_=v16[:, 0 : NT - 1]
                        )
                    exp_tile(t, d_tiles.pop(t))

                # ---- epilogue: v = exp(0.01*r) / sumw ------------------
                nc.vector.reciprocal(rs16[:, NT - 1 : NT], sumw16[:, NT - 1 : NT])
                nc.gpsimd.tensor_mul(
                    v16[:, NT - 1 : NT], maxw16[:, NT - 1 : NT],
                    rs16[:, NT - 1 : NT],
                )
                nc.sync.dma_start(
                    out=v_d[:, NT - 1 : NT], in_=v16[:, NT - 1 : NT]
                )

    nc.compile()
    return nc


_NC = None


def _get_nc():
    global _NC
    if _NC is None:
        _NC = build_nc()
    return _NC


def make_in_maps(X, Y):
    """Per-core fp16 inputs. Y columns permuted to [own-half | other-half]."""
    in_maps = []
    for c in range(N_CORES):
        b, h = c // 2, c % 2
        xs = np.ascontiguousarray(X[b][:, h * HALF : (h + 1) * HALF]).astype(
            np.float16
        )
        ys = np.ascontiguousarray(
            np.concatenate(
                [
                    Y[b][:, h * HALF : (h + 1) * HALF],
                    Y[b][:, (1 - h) * HALF : (2 - h) * HALF],
                ],
                axis=1,
            )
        ).astype(np.float16)
        in_maps.append({"x": xs, "y": ys})
    return in_maps


def finish_host(results):
    """results: list of 8 per-core dicts with 'v' [128, NT]."""
    cx = np.zeros(B, dtype=np.float64)
    for c in range(N_CORES):
        cx[c // 2] += results[c]["v"].astype(np.float64).sum()
    cx /= M
    return np.float32(np.mean(-np.log(cx)))


def run(X_features, Y_features, trace=False, tmpdir=None):
    X = np.asarray(X_features, dtype=np.float32).reshape(B, C, M)
    Y = np.asarray(Y_features, dtype=np.float32).reshape(B, C, M)
    nc = _get_nc()
    res = run_bass_kernel_spmd(
        nc, make_in_maps(X, Y), list(range(N_CORES)), trace=trace, tmpdir=tmpdir
    )
    return finish_host(res.results), res


def kernel(X_features, Y_features):
    loss, _ = run(X_features, Y_features)
    return loss
